# revision 6
# baseline (speedup 1.0000x reference)
"""NeRF MLP (131072 x 90 -> 131072 x 4) on 8 Trainium2 NeuronCores.

Strategy: pure data parallel over the ray-sample dim (16384 rows/core),
weights replicated. On-chip layout is feature-major ([feature, batch]):
the host pre-transposes x so no on-device transpose is needed, and each
Linear is computed as W.T @ actT with the weight as the stationary
matmul operand ([fan_in(K), fan_out(M)] natural layout), accumulating
K-chunks of 128 into a PSUM bank, batch tile N=512 columns.
Bias+ReLU are fused into the PSUM->SBUF drain on alternating
Scalar(ACT)/Vector(DVE) engines. Matmuls run as float32r (full PE rate
at N>=256, ~fp32 precision). Two batch tiles are interleaved at layer
granularity so the PE works on tile B while tile A's activations drain.
"""

import numpy as np

import concourse.bass as bass
import concourse.mybir as mybir
import concourse.tile as tile
from concourse import bacc, bass_utils

POS, VIEW = 63, 27
NTOT = 131072
NCORES = 8
NCORE = NTOT // NCORES  # 16384
TN = 512  # batch tile (matmul free dim; one PSUM bank of f32)
P = 128
F32 = mybir.dt.float32
F32R = mybir.dt.float32r
AF = mybir.ActivationFunctionType
ALU = mybir.AluOpType


def _r(ap):
    return ap if ap.dtype == F32R else ap.bitcast(F32R)


def build_nc(ncore=NCORE, repeat=1):
    """Build the Bass program for one core processing `ncore` samples."""
    nt = ncore // TN
    nc = bacc.Bacc(
        "TRN2", target_bir_lowering=False, debug=False, enable_asserts=False
    )

    xT = nc.dram_tensor("xT", [POS + VIEW, ncore], F32R, kind="ExternalInput")
    outT = nc.dram_tensor("outT", [4, ncore], F32, kind="ExternalOutput")

    # weights, host-side pre-chunked: [128, k_chunks, fan_out]; small first-
    # layer slices stay natural [K, fan_out]
    wspec = {
        "d1w": [POS, 256],
        "d2w": [P, 2, 256],
        "d3w": [P, 2, 256],
        "d4w": [P, 2, 256],
        "e1pw": [POS, 256],
        "e1hw": [P, 2, 256],
        "e2w": [P, 2, 256],
        "e3w": [P, 2, 256],
        "e4w": [P, 2, 256],
        "e5fw": [P, 2, 256],
        "e5dw": [P, 2, 1],
        "c1vw": [VIEW, 256],
        "c1fw": [P, 2, 256],
        "c2w": [P, 2, 3],
        # biases, host-side pre-chunked to [128, chunks] / [p, 1]
        "d1b": [P, 2],
        "d2b": [P, 2],
        "d3b": [P, 2],
        "d4b": [P, 2],
        "e1b": [P, 2],
        "e2b": [P, 2],
        "e3b": [P, 2],
        "e4b": [P, 2],
        "e5fb": [P, 2],
        "e5db": [1, 1],
        "c1b": [P, 2],
        "c2b": [3, 1],
    }
    dram = {k: nc.dram_tensor(k, v, F32R if k.endswith("w") else F32, kind="ExternalInput")
            for k, v in wspec.items()}

    with tile.TileContext(nc) as tc:
        with (
            tc.tile_pool(name="w", bufs=1) as wpool,
            tc.tile_pool(name="act", bufs=2) as apool,
            tc.tile_pool(name="xin", bufs=3) as xpool,
            tc.tile_pool(name="out", bufs=3) as opool,
            tc.tile_pool(name="psum", bufs=7, space="PSUM") as pspool,
        ):
            sb = {}
            for k, shp in wspec.items():
                t = wpool.tile(shp, F32R if k.endswith("w") else F32, tag=k,
                               name=f"sb_{k}")
                nc.sync.dma_start(t[:], dram[k][:])
                sb[k] = t

            # round-robin engine picker for PSUM->SBUF drain (bias [+ relu])
            eng_ctr = [0]

            def store(ps_ap, dest_ap, bias_ap, relu):
                use_act = (eng_ctr[0] % 2) == 0
                eng_ctr[0] += 1
                if use_act:
                    nc.scalar.activation(
                        dest_ap, ps_ap, AF.Relu if relu else AF.Identity,
                        bias=bias_ap,
                    )
                elif relu:
                    nc.vector.tensor_scalar(
                        dest_ap, ps_ap, bias_ap, 0.0, op0=ALU.add, op1=ALU.max
                    )
                else:
                    nc.vector.tensor_scalar_add(dest_ap, ps_ap, bias_ap)

            def layer256(wk, rhs_list, bk, out_t, relu, t):
                """fanout-256 layer: wk = list over k-chunks of [K,256] APs."""
                nk = len(wk)
                for m in (0, 1):
                    ps = pspool.tile([P, TN], F32, tag="ps", name=f"ps_{t}_{m}")
                    for k, (w, rhs) in enumerate(zip(wk, rhs_list)):
                        nc.tensor.matmul(
                            ps[:], _r(w[:, m * P:(m + 1) * P]), _r(rhs),
                            start=(k == 0), stop=(k == nk - 1),
                        )
                    store(ps[:], out_t[:, m, :], bk[:, m:m + 1], relu)

            def tile_gen(t, col0):
                post = xpool.tile([POS, TN], F32R, tag="post", name=f"post{t}")
                viewt = xpool.tile([VIEW, TN], F32R, tag="viewt", name=f"viewt{t}")
                nc.sync.dma_start(post[:], xT[:POS, col0:col0 + TN])
                nc.sync.dma_start(viewt[:], xT[POS:, col0:col0 + TN])
                pos = post[:]
                view = viewt[:]
                yield

                h1 = apool.tile([P, 2, TN], F32R, tag="h1", name=f"h1_{t}")
                layer256([sb["d1w"][:]], [pos], sb["d1b"], h1, True, t)
                yield
                h2 = apool.tile([P, 2, TN], F32R, tag="h2", name=f"h2_{t}")
                layer256([sb["d2w"][:, 0, :], sb["d2w"][:, 1, :]],
                         [h1[:, 0, :], h1[:, 1, :]], sb["d2b"], h2, True, t)
                yield
                h3 = apool.tile([P, 2, TN], F32R, tag="h3", name=f"h3_{t}")
                layer256([sb["d3w"][:, 0, :], sb["d3w"][:, 1, :]],
                         [h2[:, 0, :], h2[:, 1, :]], sb["d3b"], h3, True, t)
                yield
                h4 = apool.tile([P, 2, TN], F32R, tag="h4", name=f"h4_{t}")
                layer256([sb["d4w"][:, 0, :], sb["d4w"][:, 1, :]],
                         [h3[:, 0, :], h3[:, 1, :]], sb["d4b"], h4, True, t)
                yield
                g1 = apool.tile([P, 2, TN], F32R, tag="g1", name=f"g1_{t}")
                layer256([sb["e1pw"][:], sb["e1hw"][:, 0, :], sb["e1hw"][:, 1, :]],
                         [pos, h4[:, 0, :], h4[:, 1, :]], sb["e1b"], g1, True, t)
                yield
                g2 = apool.tile([P, 2, TN], F32R, tag="g2", name=f"g2_{t}")
                layer256([sb["e2w"][:, 0, :], sb["e2w"][:, 1, :]],
                         [g1[:, 0, :], g1[:, 1, :]], sb["e2b"], g2, True, t)
                yield
                g3 = apool.tile([P, 2, TN], F32R, tag="g3", name=f"g3_{t}")
                layer256([sb["e3w"][:, 0, :], sb["e3w"][:, 1, :]],
                         [g2[:, 0, :], g2[:, 1, :]], sb["e3b"], g3, True, t)
                yield
                g4 = apool.tile([P, 2, TN], F32R, tag="g4", name=f"g4_{t}")
                layer256([sb["e4w"][:, 0, :], sb["e4w"][:, 1, :]],
                         [g3[:, 0, :], g3[:, 1, :]], sb["e4b"], g4, True, t)
                yield

                f = apool.tile([P, 2, TN], F32R, tag="f", name=f"f_{t}")
                layer256([sb["e5fw"][:, 0, :], sb["e5fw"][:, 1, :]],
                         [g4[:, 0, :], g4[:, 1, :]], sb["e5fb"], f, False, t)
                psd = pspool.tile([P, TN], F32, tag="ps", name=f"psd_{t}")
                for k in (0, 1):
                    nc.tensor.matmul(
                        psd[0:1, :], _r(sb["e5dw"][:, k, :]), _r(g4[:, k, :]),
                        start=(k == 0), stop=(k == 1),
                    )
                denst = opool.tile([1, TN], F32, tag="denst", name=f"denst_{t}")
                store(psd[0:1, :], denst[:], sb["e5db"][0:1, 0:1], False)
                nc.sync.dma_start(outT[3:4, col0:col0 + TN], denst[:])
                yield
                cc = apool.tile([P, 2, TN], F32R, tag="cc", name=f"cc_{t}")
                layer256([sb["c1vw"][:], sb["c1fw"][:, 0, :], sb["c1fw"][:, 1, :]],
                         [view, f[:, 0, :], f[:, 1, :]], sb["c1b"], cc, True, t)
                yield
                psc = pspool.tile([P, TN], F32, tag="ps", name=f"psc_{t}")
                for k in (0, 1):
                    nc.tensor.matmul(
                        psc[0:3, :], _r(sb["c2w"][:, k, :]), _r(cc[:, k, :]),
                        start=(k == 0), stop=(k == 1),
                    )
                rgbt = opool.tile([3, TN], F32, tag="rgbt", name=f"rgbt_{t}")
                store(psc[0:3, :], rgbt[:], sb["c2b"][:, 0:1], False)
                nc.sync.dma_start(outT[0:3, col0:col0 + TN], rgbt[:])
                yield

            for _rep in range(repeat):
                for pair in range(nt // 2):
                    ta, tb = 2 * pair, 2 * pair + 1
                    ga = tile_gen(ta, ta * TN)
                    gb = tile_gen(tb, tb * TN)
                    for _ in zip(ga, gb):
                        pass
    nc.compile()
    return nc


def shard_inputs(inputs, ncore=NCORE, ncores=NCORES):
    """Host-side shard + layout transform to the NEFF's input tensors."""
    x = np.asarray(inputs["x"], dtype=np.float32)

    def t2(w):  # [256, M] -> [128, 2, M]
        return np.ascontiguousarray(
            np.asarray(w, np.float32).reshape(2, P, -1).transpose(1, 0, 2))

    def tb(b):  # [256] -> [128, 2]
        return np.ascontiguousarray(np.asarray(b, np.float32).reshape(2, P).T)

    i = {k: np.asarray(v, np.float32) for k, v in inputs.items()}
    shared = {
        "d1w": np.ascontiguousarray(i["d1_w"]),
        "d2w": t2(i["d2_w"]), "d3w": t2(i["d3_w"]), "d4w": t2(i["d4_w"]),
        "e1pw": np.ascontiguousarray(i["e1_w"][:POS]),
        "e1hw": t2(i["e1_w"][POS:]),
        "e2w": t2(i["e2_w"]), "e3w": t2(i["e3_w"]), "e4w": t2(i["e4_w"]),
        "e5fw": t2(i["e5_w"][:, 1:]),
        "e5dw": t2(i["e5_w"][:, :1]),
        "c1vw": np.ascontiguousarray(i["c1_w"][:VIEW]),
        "c1fw": t2(i["c1_w"][VIEW:]),
        "c2w": t2(i["c2_w"]),
        "d1b": tb(i["d1_b"]), "d2b": tb(i["d2_b"]),
        "d3b": tb(i["d3_b"]), "d4b": tb(i["d4_b"]),
        "e1b": tb(i["e1_b"]), "e2b": tb(i["e2_b"]),
        "e3b": tb(i["e3_b"]), "e4b": tb(i["e4_b"]),
        "e5fb": tb(i["e5_b"][1:]),
        "e5db": np.ascontiguousarray(i["e5_b"][:1].reshape(1, 1)),
        "c1b": tb(i["c1_b"]),
        "c2b": np.ascontiguousarray(i["c2_b"].reshape(3, 1)),
    }
    in_maps = []
    for c in range(ncores):
        xc = np.ascontiguousarray(x[c * ncore:(c + 1) * ncore, :].T)
        in_maps.append({"xT": xc, **shared})
    return in_maps


LAST_RESULTS = None
_NC_CACHE = {}


def _run(inputs, repeat=1):
    global LAST_RESULTS
    key = (NCORE, repeat)
    if key not in _NC_CACHE:
        _NC_CACHE[key] = build_nc(NCORE, repeat)
    nc = _NC_CACHE[key]
    in_maps = shard_inputs(inputs)
    import time
    t0 = time.time()
    res = bass_utils.run_bass_kernel_spmd(nc, in_maps, core_ids=list(range(NCORES)))
    dt = time.time() - t0
    LAST_RESULTS = res
    out = np.concatenate([res.results[c]["outT"] for c in range(NCORES)], axis=1)
    return np.ascontiguousarray(out.T).astype(np.float32, copy=False), dt


def kernel(**inputs):
    return _run(inputs, 1)[0]


# revision 13
# speedup vs baseline: 4.4782x; 4.4782x over previous
"""NeRF MLP (131072 x 90 -> 131072 x 4) on 8 Trainium2 NeuronCores.

Strategy: pure data parallel over the ray-sample dim (16384 rows/core),
weights replicated. On-chip layout is feature-major ([feature, batch]):
the host pre-transposes x so no on-device transpose is needed, and each
Linear is computed as W.T @ actT with the weight as the stationary
matmul operand ([fan_in(K), fan_out(M)] natural layout), accumulating
K-chunks of 128 into a PSUM bank, batch tile N=512 columns.
Bias+ReLU are fused into the PSUM->SBUF drain on alternating
Scalar(ACT)/Vector(DVE) engines. Matmuls run as float32r (full PE rate
at N>=256, ~fp32 precision). Two batch tiles are interleaved at layer
granularity so the PE works on tile B while tile A's activations drain.

PE-array packing: the K<=64 input chunks (d1: K=63, e1 pos skip: K=63,
c1 view: K=27) run both fan-out halves concurrently in disjoint PE row
groups (inputs duplicated at partition offset 64/32); the M<=32 heads
(density M=1, rgb M=3) of the two interleaved tiles run concurrently in
disjoint PE column groups.
"""

import numpy as np

import concourse.bass as bass
import concourse.mybir as mybir
import concourse.tile as tile
from concourse import bacc, bass_utils

POS, VIEW = 63, 27
NTOT = 131072
NCORES = 8
NCORE = NTOT // NCORES  # 16384
TN = 512  # batch tile (matmul free dim; one PSUM bank of f32)
P = 128
F32 = mybir.dt.float32
F32R = mybir.dt.float32r
AF = mybir.ActivationFunctionType
ALU = mybir.AluOpType

PACK_D1 = True
PACK_E1 = True
PACK_C1 = True


def build_nc(ncore=NCORE, repeat=1):
    """Build the Bass program for one core processing `ncore` samples."""
    nt = ncore // TN
    nc = bacc.Bacc(
        "TRN2", target_bir_lowering=False, debug=False, enable_asserts=False
    )

    xT = nc.dram_tensor("xT", [POS + VIEW, ncore], F32R, kind="ExternalInput")
    outT = nc.dram_tensor("outT", [4, ncore], F32, kind="ExternalOutput")

    # weights, host-side pre-chunked: [128, k_chunks, fan_out]. K<=64 first-
    # layer chunks are row-group packed: both fan-out halves stacked in the
    # partition dim ([p, 128]).
    wspec = {
        "d1w": [P, P],       # rows 0:63 -> W[:, :128], rows 64:127 -> W[:, 128:]
        "d1wU": [POS, 256],
        "e1pwU": [POS, 256],
        "c1vwU": [VIEW, 256],
        "d2w": [P, 2, 256],
        "d3w": [P, 2, 256],
        "d4w": [P, 2, 256],
        "e1pw": [P, P],      # packed like d1w
        "e1hw": [P, 2, 256],
        "e2w": [P, 2, 256],
        "e3w": [P, 2, 256],
        "e4w": [P, 2, 256],
        "e5fw": [P, 2, 256],
        "e5dw": [P, 2, 1],
        "c1vw": [64, P],     # rows 0:27 -> W[:, :128], rows 32:59 -> W[:, 128:]
        "c1fw": [P, 2, 256],
        "c2w": [P, 2, 3],
        # biases, host-side pre-chunked to [128, chunks]; the tiny heads are
        # replicated at partition offset 32 for the pair-packed drains
        "d1b": [P, 2],
        "d2b": [P, 2],
        "d3b": [P, 2],
        "d4b": [P, 2],
        "e1b": [P, 2],
        "e2b": [P, 2],
        "e3b": [P, 2],
        "e4b": [P, 2],
        "e5fb": [P, 2],
        "e5db": [1, 1],
        "c1b": [P, 2],
        "c2b": [3, 1],
    }
    dram = {k: nc.dram_tensor(k, v, F32 if k.endswith("b") else F32R, kind="ExternalInput")
            for k, v in wspec.items()}

    with tile.TileContext(nc) as tc:
        with (
            tc.tile_pool(name="w", bufs=1) as wpool,
            tc.tile_pool(name="act", bufs=2) as apool,
            tc.tile_pool(name="xin", bufs=3) as xpool,
            tc.tile_pool(name="out", bufs=3) as opool,
            tc.tile_pool(name="psum", bufs=7, space="PSUM") as pspool,
        ):
            sb = {}
            for k, shp in wspec.items():
                t = wpool.tile(shp, F32 if k.endswith("b") else F32R, tag=k,
                               name=f"sb_{k}")
                nc.sync.dma_start(t[:], dram[k][:])
                sb[k] = t

            # round-robin engine picker for PSUM->SBUF drain (bias [+ relu])
            eng_ctr = [0]

            def store(ps_ap, dest_ap, bias_ap, relu):
                use_act = (eng_ctr[0] % 2) == 0
                eng_ctr[0] += 1
                if use_act:
                    nc.scalar.activation(
                        dest_ap, ps_ap, AF.Relu if relu else AF.Identity,
                        bias=bias_ap,
                    )
                elif relu:
                    nc.vector.tensor_scalar(
                        dest_ap, ps_ap, bias_ap, 0.0, op0=ALU.add, op1=ALU.max
                    )
                else:
                    nc.vector.tensor_scalar_add(dest_ap, ps_ap, bias_ap)

            def layer256(wk, rhs_list, bk, out_t, relu, t):
                """fanout-256 layer: wk = list over k-chunks of [K,256] APs."""
                nk = len(wk)
                for m in (0, 1):
                    ps = pspool.tile([P, TN], F32, tag="ps", name=f"ps_{t}_{m}")
                    for k, (w, rhs) in enumerate(zip(wk, rhs_list)):
                        nc.tensor.matmul(
                            ps[:], w[:, m * P:(m + 1) * P], rhs,
                            start=(k == 0), stop=(k == nk - 1),
                        )
                    store(ps[:], out_t[:, m, :], bk[:, m:m + 1], relu)

            def packed_layer(wpk, rhs2, lo_hi, whk, rhs_h, bk, out_t, relu, t):
                """K<=64 chunk packed in two row groups (+ optional 256-K
                chunks). wpk: packed weight tile; rhs2: duplicated input
                tile; lo_hi: ((alo,ahi),(blo,bhi)) partition ranges of the
                two row groups; whk: [128,2,256] weight or None; rhs_h:
                [128,2,TN] input tile or None."""
                pss = []
                for m, (lo, hi) in enumerate(lo_hi):
                    ps = pspool.tile([P, TN], F32, tag="ps", name=f"pp_{t}_{m}")
                    nc.tensor.matmul(
                        ps[:], wpk[lo:hi, :], rhs2[lo:hi, :],
                        start=True, stop=whk is None,
                        skip_group_check=whk is not None,
                    )
                    pss.append(ps)
                if whk is not None:
                    for m in (0, 1):
                        for k in (0, 1):
                            nc.tensor.matmul(
                                pss[m][:], whk[:, k, m * P:(m + 1) * P],
                                rhs_h[:, k, :],
                                start=False, stop=(k == 1),
                                skip_group_check=True,
                            )
                for m in (0, 1):
                    store(pss[m][:], out_t[:, m, :], bk[:, m:m + 1], relu)

            def tile_gen(t, col0, ctx):
                pos2 = xpool.tile([P, TN], F32R, tag="pos2", name=f"pos2_{t}")
                view2 = xpool.tile([64, TN], F32R, tag="view2", name=f"view2_{t}")
                nc.sync.dma_start(pos2[0:POS, :], xT[:POS, col0:col0 + TN])
                nc.sync.dma_start(pos2[64:64 + POS, :], xT[:POS, col0:col0 + TN])
                nc.sync.dma_start(view2[0:VIEW, :], xT[POS:, col0:col0 + TN])
                nc.sync.dma_start(view2[32:32 + VIEW, :], xT[POS:, col0:col0 + TN])
                ctx["col0"] = col0
                yield

                h1 = apool.tile([P, 2, TN], F32R, tag="h1", name=f"h1_{t}")
                if PACK_D1:
                    packed_layer(sb["d1w"], pos2, ((0, POS), (64, 64 + POS)),
                                 None, None, sb["d1b"], h1, True, t)
                else:
                    layer256([sb["d1wU"][:]], [pos2[0:POS, :]], sb["d1b"],
                             h1, True, t)
                yield
                h2 = apool.tile([P, 2, TN], F32R, tag="h2", name=f"h2_{t}")
                layer256([sb["d2w"][:, 0, :], sb["d2w"][:, 1, :]],
                         [h1[:, 0, :], h1[:, 1, :]], sb["d2b"], h2, True, t)
                yield
                h3 = apool.tile([P, 2, TN], F32R, tag="h3", name=f"h3_{t}")
                layer256([sb["d3w"][:, 0, :], sb["d3w"][:, 1, :]],
                         [h2[:, 0, :], h2[:, 1, :]], sb["d3b"], h3, True, t)
                yield
                h4 = apool.tile([P, 2, TN], F32R, tag="h4", name=f"h4_{t}")
                layer256([sb["d4w"][:, 0, :], sb["d4w"][:, 1, :]],
                         [h3[:, 0, :], h3[:, 1, :]], sb["d4b"], h4, True, t)
                yield
                g1 = apool.tile([P, 2, TN], F32R, tag="g1", name=f"g1_{t}")
                if PACK_E1:
                    packed_layer(sb["e1pw"], pos2, ((0, POS), (64, 64 + POS)),
                                 sb["e1hw"], h4, sb["e1b"], g1, True, t)
                else:
                    layer256([sb["e1pwU"][:], sb["e1hw"][:, 0, :],
                              sb["e1hw"][:, 1, :]],
                             [pos2[0:POS, :], h4[:, 0, :], h4[:, 1, :]],
                             sb["e1b"], g1, True, t)
                yield
                g2 = apool.tile([P, 2, TN], F32R, tag="g2", name=f"g2_{t}")
                layer256([sb["e2w"][:, 0, :], sb["e2w"][:, 1, :]],
                         [g1[:, 0, :], g1[:, 1, :]], sb["e2b"], g2, True, t)
                yield
                g3 = apool.tile([P, 2, TN], F32R, tag="g3", name=f"g3_{t}")
                layer256([sb["e3w"][:, 0, :], sb["e3w"][:, 1, :]],
                         [g2[:, 0, :], g2[:, 1, :]], sb["e3b"], g3, True, t)
                yield
                g4 = apool.tile([P, 2, TN], F32R, tag="g4", name=f"g4_{t}")
                layer256([sb["e4w"][:, 0, :], sb["e4w"][:, 1, :]],
                         [g3[:, 0, :], g3[:, 1, :]], sb["e4b"], g4, True, t)
                yield
                f = apool.tile([P, 2, TN], F32R, tag="f", name=f"f_{t}")
                layer256([sb["e5fw"][:, 0, :], sb["e5fw"][:, 1, :]],
                         [g4[:, 0, :], g4[:, 1, :]], sb["e5fb"], f, False, t)
                psd = pspool.tile([P, TN], F32, tag="ps", name=f"psd_{t}")
                for k in (0, 1):
                    nc.tensor.matmul(
                        psd[0:1, :], sb["e5dw"][:, k, :], g4[:, k, :],
                        start=(k == 0), stop=(k == 1),
                    )
                denst = opool.tile([1, TN], F32, tag="denst", name=f"denst_{t}")
                store(psd[0:1, :], denst[:], sb["e5db"][0:1, 0:1], False)
                nc.sync.dma_start(outT[3:4, col0:col0 + TN], denst[:])
                yield
                cc = apool.tile([P, 2, TN], F32R, tag="cc", name=f"cc_{t}")
                if PACK_C1:
                    packed_c1(view2, f, cc, t)
                else:
                    layer256([sb["c1vwU"][:], sb["c1fw"][:, 0, :],
                              sb["c1fw"][:, 1, :]],
                             [view2[0:VIEW, :], f[:, 0, :], f[:, 1, :]],
                             sb["c1b"], cc, True, t)
                psc = pspool.tile([P, TN], F32, tag="ps", name=f"psc_{t}")
                for k in (0, 1):
                    nc.tensor.matmul(
                        psc[0:3, :], sb["c2w"][:, k, :], cc[:, k, :],
                        start=(k == 0), stop=(k == 1),
                    )
                rgbt = opool.tile([3, TN], F32, tag="rgbt", name=f"rgbt_{t}")
                store(psc[0:3, :], rgbt[:], sb["c2b"][:, 0:1], False)
                nc.sync.dma_start(outT[0:3, col0:col0 + TN], rgbt[:])
                yield

            def packed_c1(view2, f, cc, t):
                pss = []
                for m, (lo, hi) in enumerate(((0, VIEW), (32, 32 + VIEW))):
                    ps = pspool.tile([P, TN], F32, tag="ps", name=f"pc1_{t}_{m}")
                    nc.tensor.matmul(
                        ps[:], sb["c1vw"][lo:hi, :], view2[lo:hi, :],
                        start=True, stop=False, skip_group_check=True,
                    )
                    pss.append(ps)
                for m in (0, 1):
                    for k in (0, 1):
                        nc.tensor.matmul(
                            pss[m][:], sb["c1fw"][:, k, m * P:(m + 1) * P],
                            f[:, k, :],
                            start=False, stop=(k == 1), skip_group_check=True,
                        )
                for m in (0, 1):
                    store(pss[m][:], cc[:, m, :], sb["c1b"][:, m:m + 1], True)

            for _rep in range(repeat):
                for pair in range(nt // 2):
                    ta, tb = 2 * pair, 2 * pair + 1
                    ga = tile_gen(ta, ta * TN, {})
                    gb = tile_gen(tb, tb * TN, {})
                    for _ in zip(ga, gb):
                        pass
    nc.compile()
    return nc


def shard_inputs(inputs, ncore=NCORE, ncores=NCORES):
    """Host-side shard + layout transform to the NEFF's input tensors."""
    x = np.asarray(inputs["x"], dtype=np.float32)

    def t2(w):  # [256, M] -> [128, 2, M]
        return np.ascontiguousarray(
            np.asarray(w, np.float32).reshape(2, P, -1).transpose(1, 0, 2))

    def tb(b):  # [256] -> [128, 2]
        return np.ascontiguousarray(np.asarray(b, np.float32).reshape(2, P).T)

    def pack_rows(w, off, rows):  # [K, 256] -> [rows, 128] two row groups
        out = np.zeros((rows, P), np.float32)
        out[:w.shape[0], :] = w[:, :P]
        out[off:off + w.shape[0], :] = w[:, P:]
        return np.ascontiguousarray(out)

    i = {k: np.asarray(v, np.float32) for k, v in inputs.items()}
    e5db = np.ascontiguousarray(i["e5_b"][:1].reshape(1, 1))
    c2b = np.ascontiguousarray(i["c2_b"].reshape(3, 1))
    shared = {
        "d1w": pack_rows(i["d1_w"], 64, P),
        "d1wU": np.ascontiguousarray(i["d1_w"]),
        "e1pwU": np.ascontiguousarray(i["e1_w"][:POS]),
        "c1vwU": np.ascontiguousarray(i["c1_w"][:VIEW]),
        "d2w": t2(i["d2_w"]), "d3w": t2(i["d3_w"]), "d4w": t2(i["d4_w"]),
        "e1pw": pack_rows(i["e1_w"][:POS], 64, P),
        "e1hw": t2(i["e1_w"][POS:]),
        "e2w": t2(i["e2_w"]), "e3w": t2(i["e3_w"]), "e4w": t2(i["e4_w"]),
        "e5fw": t2(i["e5_w"][:, 1:]),
        "e5dw": t2(i["e5_w"][:, :1]),
        "c1vw": pack_rows(i["c1_w"][:VIEW], 32, 64),
        "c1fw": t2(i["c1_w"][VIEW:]),
        "c2w": t2(i["c2_w"]),
        "d1b": tb(i["d1_b"]), "d2b": tb(i["d2_b"]),
        "d3b": tb(i["d3_b"]), "d4b": tb(i["d4_b"]),
        "e1b": tb(i["e1_b"]), "e2b": tb(i["e2_b"]),
        "e3b": tb(i["e3_b"]), "e4b": tb(i["e4_b"]),
        "e5fb": tb(i["e5_b"][1:]),
        "e5db": e5db,
        "c1b": tb(i["c1_b"]),
        "c2b": c2b,
    }
    in_maps = []
    for c in range(ncores):
        xc = np.ascontiguousarray(x[c * ncore:(c + 1) * ncore, :].T)
        in_maps.append({"xT": xc, **shared})
    return in_maps


LAST_RESULTS = None
_NC_CACHE = {}


def _run(inputs, repeat=1):
    global LAST_RESULTS
    key = (NCORE, repeat)
    if key not in _NC_CACHE:
        _NC_CACHE[key] = build_nc(NCORE, repeat)
    nc = _NC_CACHE[key]
    in_maps = shard_inputs(inputs)
    import time
    t0 = time.time()
    res = bass_utils.run_bass_kernel_spmd(nc, in_maps, core_ids=list(range(NCORES)))
    dt = time.time() - t0
    LAST_RESULTS = res
    out = np.concatenate([res.results[c]["outT"] for c in range(NCORES)], axis=1)
    return np.ascontiguousarray(out.T).astype(np.float32, copy=False), dt


def kernel(**inputs):
    return _run(inputs, 1)[0]


# revision 16
# speedup vs baseline: 15856.7745x; 3540.8910x over previous
"""NeRF MLP (131072 x 90 -> 131072 x 4) on 8 Trainium2 NeuronCores.

Strategy: pure data parallel over the ray-sample dim (16384 rows/core),
weights replicated. On-chip layout is feature-major ([feature, batch]):
the host pre-transposes x so no on-device transpose is needed, and each
Linear is computed as W.T @ actT with the weight as the stationary
matmul operand ([fan_in(K), fan_out(M)] natural layout), accumulating
K-chunks of 128 into a PSUM bank, batch tile N=512 columns.
Bias+ReLU are fused into the PSUM->SBUF drain on alternating
Scalar(ACT)/Vector(DVE) engines. Matmuls run as float32r (full PE rate
at N>=256, ~fp32 precision). Three batch tiles are interleaved at layer
granularity so the PE works on other tiles while a tile's activations
drain through ACT/DVE.

PE-array row-group packing: the K<=64 input chunks (d1: K=63, e1 pos
skip: K=63, c1 view: K=27) run both fan-out halves concurrently in
disjoint PE row groups (tile_position derived from base partitions;
inputs duplicated at partition offset 64/32).
"""

import numpy as np

import concourse.bass as bass
import concourse.mybir as mybir
import concourse.tile as tile
from concourse import bacc, bass_utils

POS, VIEW = 63, 27
NTOT = 131072
NCORES = 8
NCORE = NTOT // NCORES  # 16384
TN = 512  # batch tile (matmul free dim; one PSUM bank of f32)
P = 128
F32 = mybir.dt.float32
F32R = mybir.dt.float32r
AF = mybir.ActivationFunctionType
ALU = mybir.AluOpType

PACK_D1 = True
PACK_E1 = True
PACK_C1 = True
INTERLEAVE = 3
PSUM_BUFS = 8
ACT_BUFS = 3


def build_nc(ncore=NCORE, repeat=1):
    """Build the Bass program for one core processing `ncore` samples."""
    nt = ncore // TN
    nc = bacc.Bacc(
        "TRN2", target_bir_lowering=False, debug=False, enable_asserts=False
    )

    xT = nc.dram_tensor("xT", [POS + VIEW, ncore], F32R, kind="ExternalInput")
    outT = nc.dram_tensor("outT", [4, ncore], F32, kind="ExternalOutput")

    # weights, host-side pre-chunked: [128, k_chunks, fan_out]. K<=64 first-
    # layer chunks are row-group packed: both fan-out halves stacked in the
    # partition dim ([p, 128]).
    wspec = {
        "d1w": [P, P],       # rows 0:63 -> W[:, :128], rows 64:127 -> W[:, 128:]
        "d1wU": [POS, 256],
        "e1pwU": [POS, 256],
        "c1vwU": [VIEW, 256],
        "d2w": [P, 2, 256],
        "d3w": [P, 2, 256],
        "d4w": [P, 2, 256],
        "e1pw": [P, P],      # packed like d1w
        "e1hw": [P, 2, 256],
        "e2w": [P, 2, 256],
        "e3w": [P, 2, 256],
        "e4w": [P, 2, 256],
        "e5fw": [P, 2, 256],
        "e5dw": [P, 2, 1],
        "c1vw": [64, P],     # rows 0:27 -> W[:, :128], rows 32:59 -> W[:, 128:]
        "c1fw": [P, 2, 256],
        "c2w": [P, 2, 3],
        # biases, host-side pre-chunked to [128, chunks]; the tiny heads are
        # replicated at partition offset 32 for the pair-packed drains
        "d1b": [P, 2],
        "d2b": [P, 2],
        "d3b": [P, 2],
        "d4b": [P, 2],
        "e1b": [P, 2],
        "e2b": [P, 2],
        "e3b": [P, 2],
        "e4b": [P, 2],
        "e5fb": [P, 2],
        "e5db": [1, 1],
        "c1b": [P, 2],
        "c2b": [3, 1],
    }
    dram = {k: nc.dram_tensor(k, v, F32 if k.endswith("b") else F32R, kind="ExternalInput")
            for k, v in wspec.items()}

    with tile.TileContext(nc) as tc:
        with (
            tc.tile_pool(name="w", bufs=1) as wpool,
            tc.tile_pool(name="act", bufs=ACT_BUFS) as apool,
            tc.tile_pool(name="xin", bufs=INTERLEAVE + 1) as xpool,
            tc.tile_pool(name="out", bufs=INTERLEAVE + 1) as opool,
            tc.tile_pool(name="psum", bufs=PSUM_BUFS, space="PSUM") as pspool,
        ):
            sb = {}
            for k, shp in wspec.items():
                t = wpool.tile(shp, F32 if k.endswith("b") else F32R, tag=k,
                               name=f"sb_{k}")
                nc.sync.dma_start(t[:], dram[k][:])
                sb[k] = t

            # round-robin engine picker for PSUM->SBUF drain (bias [+ relu])
            eng_ctr = [0]

            def store(ps_ap, dest_ap, bias_ap, relu):
                use_act = (eng_ctr[0] % 2) == 0
                eng_ctr[0] += 1
                if use_act:
                    nc.scalar.activation(
                        dest_ap, ps_ap, AF.Relu if relu else AF.Identity,
                        bias=bias_ap,
                    )
                elif relu:
                    nc.vector.tensor_scalar(
                        dest_ap, ps_ap, bias_ap, 0.0, op0=ALU.add, op1=ALU.max
                    )
                else:
                    nc.vector.tensor_scalar_add(dest_ap, ps_ap, bias_ap)

            def layer256(wk, rhs_list, bk, out_t, relu, t):
                """fanout-256 layer: wk = list over k-chunks of [K,256] APs."""
                nk = len(wk)
                for m in (0, 1):
                    ps = pspool.tile([P, TN], F32, tag="ps", name=f"ps_{t}_{m}")
                    for k, (w, rhs) in enumerate(zip(wk, rhs_list)):
                        nc.tensor.matmul(
                            ps[:], w[:, m * P:(m + 1) * P], rhs,
                            start=(k == 0), stop=(k == nk - 1),
                        )
                    store(ps[:], out_t[:, m, :], bk[:, m:m + 1], relu)

            def packed_layer(wpk, rhs2, lo_hi, whk, rhs_h, bk, out_t, relu, t):
                """K<=64 chunk packed in two row groups (+ optional 256-K
                chunks). wpk: packed weight tile; rhs2: duplicated input
                tile; lo_hi: ((alo,ahi),(blo,bhi)) partition ranges of the
                two row groups; whk: [128,2,256] weight or None; rhs_h:
                [128,2,TN] input tile or None."""
                pss = []
                for m, (lo, hi) in enumerate(lo_hi):
                    ps = pspool.tile([P, TN], F32, tag="ps", name=f"pp_{t}_{m}")
                    nc.tensor.matmul(
                        ps[:], wpk[lo:hi, :], rhs2[lo:hi, :],
                        start=True, stop=whk is None,
                        skip_group_check=whk is not None,
                    )
                    pss.append(ps)
                if whk is not None:
                    for m in (0, 1):
                        for k in (0, 1):
                            nc.tensor.matmul(
                                pss[m][:], whk[:, k, m * P:(m + 1) * P],
                                rhs_h[:, k, :],
                                start=False, stop=(k == 1),
                                skip_group_check=True,
                            )
                for m in (0, 1):
                    store(pss[m][:], out_t[:, m, :], bk[:, m:m + 1], relu)

            def tile_gen(t, col0, ctx):
                pos2 = xpool.tile([P, TN], F32R, tag="pos2", name=f"pos2_{t}")
                view2 = xpool.tile([64, TN], F32R, tag="view2", name=f"view2_{t}")
                nc.sync.dma_start(pos2[0:POS, :], xT[:POS, col0:col0 + TN])
                nc.sync.dma_start(pos2[64:64 + POS, :], xT[:POS, col0:col0 + TN])
                nc.sync.dma_start(view2[0:VIEW, :], xT[POS:, col0:col0 + TN])
                nc.sync.dma_start(view2[32:32 + VIEW, :], xT[POS:, col0:col0 + TN])
                ctx["col0"] = col0
                yield

                h1 = apool.tile([P, 2, TN], F32R, tag="h1", name=f"h1_{t}")
                if PACK_D1:
                    packed_layer(sb["d1w"], pos2, ((0, POS), (64, 64 + POS)),
                                 None, None, sb["d1b"], h1, True, t)
                else:
                    layer256([sb["d1wU"][:]], [pos2[0:POS, :]], sb["d1b"],
                             h1, True, t)
                yield
                h2 = apool.tile([P, 2, TN], F32R, tag="h2", name=f"h2_{t}")
                layer256([sb["d2w"][:, 0, :], sb["d2w"][:, 1, :]],
                         [h1[:, 0, :], h1[:, 1, :]], sb["d2b"], h2, True, t)
                yield
                h3 = apool.tile([P, 2, TN], F32R, tag="h3", name=f"h3_{t}")
                layer256([sb["d3w"][:, 0, :], sb["d3w"][:, 1, :]],
                         [h2[:, 0, :], h2[:, 1, :]], sb["d3b"], h3, True, t)
                yield
                h4 = apool.tile([P, 2, TN], F32R, tag="h4", name=f"h4_{t}")
                layer256([sb["d4w"][:, 0, :], sb["d4w"][:, 1, :]],
                         [h3[:, 0, :], h3[:, 1, :]], sb["d4b"], h4, True, t)
                yield
                g1 = apool.tile([P, 2, TN], F32R, tag="g1", name=f"g1_{t}")
                if PACK_E1:
                    packed_layer(sb["e1pw"], pos2, ((0, POS), (64, 64 + POS)),
                                 sb["e1hw"], h4, sb["e1b"], g1, True, t)
                else:
                    layer256([sb["e1pwU"][:], sb["e1hw"][:, 0, :],
                              sb["e1hw"][:, 1, :]],
                             [pos2[0:POS, :], h4[:, 0, :], h4[:, 1, :]],
                             sb["e1b"], g1, True, t)
                yield
                g2 = apool.tile([P, 2, TN], F32R, tag="g2", name=f"g2_{t}")
                layer256([sb["e2w"][:, 0, :], sb["e2w"][:, 1, :]],
                         [g1[:, 0, :], g1[:, 1, :]], sb["e2b"], g2, True, t)
                yield
                g3 = apool.tile([P, 2, TN], F32R, tag="g3", name=f"g3_{t}")
                layer256([sb["e3w"][:, 0, :], sb["e3w"][:, 1, :]],
                         [g2[:, 0, :], g2[:, 1, :]], sb["e3b"], g3, True, t)
                yield
                g4 = apool.tile([P, 2, TN], F32R, tag="g4", name=f"g4_{t}")
                layer256([sb["e4w"][:, 0, :], sb["e4w"][:, 1, :]],
                         [g3[:, 0, :], g3[:, 1, :]], sb["e4b"], g4, True, t)
                yield
                f = apool.tile([P, 2, TN], F32R, tag="f", name=f"f_{t}")
                layer256([sb["e5fw"][:, 0, :], sb["e5fw"][:, 1, :]],
                         [g4[:, 0, :], g4[:, 1, :]], sb["e5fb"], f, False, t)
                psd = pspool.tile([P, TN], F32, tag="ps", name=f"psd_{t}")
                for k in (0, 1):
                    nc.tensor.matmul(
                        psd[0:1, :], sb["e5dw"][:, k, :], g4[:, k, :],
                        start=(k == 0), stop=(k == 1),
                    )
                denst = opool.tile([1, TN], F32, tag="denst", name=f"denst_{t}")
                store(psd[0:1, :], denst[:], sb["e5db"][0:1, 0:1], False)
                nc.sync.dma_start(outT[3:4, col0:col0 + TN], denst[:])
                yield
                cc = apool.tile([P, 2, TN], F32R, tag="cc", name=f"cc_{t}")
                if PACK_C1:
                    packed_c1(view2, f, cc, t)
                else:
                    layer256([sb["c1vwU"][:], sb["c1fw"][:, 0, :],
                              sb["c1fw"][:, 1, :]],
                             [view2[0:VIEW, :], f[:, 0, :], f[:, 1, :]],
                             sb["c1b"], cc, True, t)
                psc = pspool.tile([P, TN], F32, tag="ps", name=f"psc_{t}")
                for k in (0, 1):
                    nc.tensor.matmul(
                        psc[0:3, :], sb["c2w"][:, k, :], cc[:, k, :],
                        start=(k == 0), stop=(k == 1),
                    )
                rgbt = opool.tile([3, TN], F32, tag="rgbt", name=f"rgbt_{t}")
                store(psc[0:3, :], rgbt[:], sb["c2b"][:, 0:1], False)
                nc.sync.dma_start(outT[0:3, col0:col0 + TN], rgbt[:])
                yield

            def packed_c1(view2, f, cc, t):
                pss = []
                for m, (lo, hi) in enumerate(((0, VIEW), (32, 32 + VIEW))):
                    ps = pspool.tile([P, TN], F32, tag="ps", name=f"pc1_{t}_{m}")
                    nc.tensor.matmul(
                        ps[:], sb["c1vw"][lo:hi, :], view2[lo:hi, :],
                        start=True, stop=False, skip_group_check=True,
                    )
                    pss.append(ps)
                for m in (0, 1):
                    for k in (0, 1):
                        nc.tensor.matmul(
                            pss[m][:], sb["c1fw"][:, k, m * P:(m + 1) * P],
                            f[:, k, :],
                            start=False, stop=(k == 1), skip_group_check=True,
                        )
                for m in (0, 1):
                    store(pss[m][:], cc[:, m, :], sb["c1b"][:, m:m + 1], True)

            def emit_all_tiles():
                t0 = 0
                while t0 < nt:
                    g = min(INTERLEAVE, nt - t0)
                    if nt - t0 - g == 1:
                        g += 1  # avoid a trailing singleton group
                    gens = [tile_gen(t0 + j, (t0 + j) * TN, {})
                            for j in range(g)]
                    for _ in zip(*gens):
                        pass
                    t0 += g

            if repeat == 1:
                emit_all_tiles()
            else:
                # hardware loop: same instruction stream executed `repeat`
                # times (used only for wall-clock timing amplification)
                with tc.For_i(0, repeat, 1):
                    emit_all_tiles()
    nc.compile()
    return nc


def shard_inputs(inputs, ncore=NCORE, ncores=NCORES):
    """Host-side shard + layout transform to the NEFF's input tensors."""
    x = np.asarray(inputs["x"], dtype=np.float32)

    def t2(w):  # [256, M] -> [128, 2, M]
        return np.ascontiguousarray(
            np.asarray(w, np.float32).reshape(2, P, -1).transpose(1, 0, 2))

    def tb(b):  # [256] -> [128, 2]
        return np.ascontiguousarray(np.asarray(b, np.float32).reshape(2, P).T)

    def pack_rows(w, off, rows):  # [K, 256] -> [rows, 128] two row groups
        out = np.zeros((rows, P), np.float32)
        out[:w.shape[0], :] = w[:, :P]
        out[off:off + w.shape[0], :] = w[:, P:]
        return np.ascontiguousarray(out)

    i = {k: np.asarray(v, np.float32) for k, v in inputs.items()}
    e5db = np.ascontiguousarray(i["e5_b"][:1].reshape(1, 1))
    c2b = np.ascontiguousarray(i["c2_b"].reshape(3, 1))
    shared = {
        "d1w": pack_rows(i["d1_w"], 64, P),
        "d1wU": np.ascontiguousarray(i["d1_w"]),
        "e1pwU": np.ascontiguousarray(i["e1_w"][:POS]),
        "c1vwU": np.ascontiguousarray(i["c1_w"][:VIEW]),
        "d2w": t2(i["d2_w"]), "d3w": t2(i["d3_w"]), "d4w": t2(i["d4_w"]),
        "e1pw": pack_rows(i["e1_w"][:POS], 64, P),
        "e1hw": t2(i["e1_w"][POS:]),
        "e2w": t2(i["e2_w"]), "e3w": t2(i["e3_w"]), "e4w": t2(i["e4_w"]),
        "e5fw": t2(i["e5_w"][:, 1:]),
        "e5dw": t2(i["e5_w"][:, :1]),
        "c1vw": pack_rows(i["c1_w"][:VIEW], 32, 64),
        "c1fw": t2(i["c1_w"][VIEW:]),
        "c2w": t2(i["c2_w"]),
        "d1b": tb(i["d1_b"]), "d2b": tb(i["d2_b"]),
        "d3b": tb(i["d3_b"]), "d4b": tb(i["d4_b"]),
        "e1b": tb(i["e1_b"]), "e2b": tb(i["e2_b"]),
        "e3b": tb(i["e3_b"]), "e4b": tb(i["e4_b"]),
        "e5fb": tb(i["e5_b"][1:]),
        "e5db": e5db,
        "c1b": tb(i["c1_b"]),
        "c2b": c2b,
    }
    in_maps = []
    for c in range(ncores):
        xc = np.ascontiguousarray(x[c * ncore:(c + 1) * ncore, :].T)
        in_maps.append({"xT": xc, **shared})
    return in_maps


LAST_RESULTS = None
_NC_CACHE = {}


def _run(inputs, repeat=1):
    global LAST_RESULTS
    key = (NCORE, repeat)
    if key not in _NC_CACHE:
        _NC_CACHE[key] = build_nc(NCORE, repeat)
    nc = _NC_CACHE[key]
    in_maps = shard_inputs(inputs)
    import time
    t0 = time.time()
    res = bass_utils.run_bass_kernel_spmd(nc, in_maps, core_ids=list(range(NCORES)))
    dt = time.time() - t0
    LAST_RESULTS = res
    out = np.concatenate([res.results[c]["outT"] for c in range(NCORES)], axis=1)
    return np.ascontiguousarray(out.T).astype(np.float32, copy=False), dt


def kernel(**inputs):
    return _run(inputs, 1)[0]
